# revision 19
# baseline (speedup 1.0000x reference)
"""Trainium2 Bass kernel for the BERT span-pair classifier problem.

Computes, for B=2 batches over a 252x252 span-pair grid:
    h    = relu(Ai[i] + Aj[j] + ind(i,j)*w1c + b1)        # [770] per pair
    out  = h @ W2.T + b2                                   # [36]  per pair
    out  = where(span_mask >= 1, out, 0)
    res  = log_softmax(out over the 63504 pairs)           # per (batch, label)
    return res transposed to [B, 36, L*L]

Strategy (8 NeuronCores, SPMD single program):
  - The 504 (batch, row-i) rows are distributed over 8 cores, grouped into 4
    slot segments (in-span b0, in-span b1, off-span b0, off-span b1) so that
    every core runs an identical instruction template per slot.  In-span
    segments are assigned round-robin (row = s_b + 8*k + core).
  - h is produced in [hid-chunk(128) x j] layout, bf16 end to end so the DVE
    tensor_scalar ops hit the packed 4x mode.  Per in-span slot the w1c
    window [i, e] is written as a static suffix [B0+8(+parity), e+1] from BjE
    plus a single 9-wide dynamic ACT op covering the per-core ambiguity band.
  - The 770->36 matmul streams h against stationary W2^T chunks, two chunks
    at a time in separate PE column groups (tile_position col packing); the
    two PSUM half-sums are merged and masked into fp16 s_out.
  - exp runs on the *masked* logits so invalid pairs contribute exp(0)=1
    exactly as the reference does; padding slots are corrected by a host
    constant.  The cross-core reduction is a tiny AllGather + local tree.
  - The final (out - lse) uses 4 big fp16 4x tensor_scalar ops + 4 DMAs.
"""

import math
import os
from contextlib import ExitStack

import numpy as np

import concourse.bass as bass
import concourse.bacc as bacc
import concourse.tile as tile
from concourse import mybir
from concourse._compat import with_exitstack
from concourse.bass_utils import run_bass_kernel_spmd

L = 252
HID = 768
MLP = 770
NLAB = 36
B = 2
NC = 8
KC = 6           # full 128-row hid chunks (6*128 = 768)

FP32 = mybir.dt.float32
FP16 = mybir.dt.float16
BF16 = mybir.dt.bfloat16
AF = mybir.ActivationFunctionType
ALU = mybir.AluOpType

COLPACK = True   # two W2 chunks in separate PE column groups


def _ceil2(x):
    return x + (x & 1)


def plan_slots(spans):
    """Compute the slot layout shared by host + device.

    Returns dict with per-segment (kind, batch, start_slot, nslots, rows).
    In-span segments: core c, slot p holds global row  s_b + 8*(p-start) + c.
    Off-span segments: arbitrary round-robin over the listed rows.
    """
    segs = []
    slot = 0
    for b in range(B):
        s, e = spans[b]
        n = e - s + 1
        nsl = _ceil2(math.ceil(n / NC))
        segs.append(dict(kind="in", b=b, start=slot, nslots=nsl, s=s, e=e, count=n))
        slot += nsl
    for b in range(B):
        s, e = spans[b]
        rows = [r for r in range(L) if r < s or r > e]
        nsl = _ceil2(math.ceil(len(rows) / NC))
        segs.append(dict(kind="off", b=b, start=slot, nslots=nsl, rows=rows,
                         count=len(rows)))
        slot += nsl
    nslot = slot
    assert nslot % 2 == 0
    return segs, nslot


def slot_map_for_core(segs, nslot, c):
    """-> list over slots of (batch, global_row) or None for padding."""
    m = [None] * nslot
    for sg in segs:
        for k in range(sg["nslots"]):
            idx = NC * k + c
            p = sg["start"] + k
            if idx < sg["count"]:
                if sg["kind"] == "in":
                    m[p] = (sg["b"], sg["s"] + idx)
                else:
                    m[p] = (sg["b"], sg["rows"][idx])
    return m


def build_kernel(spans, segs, nslot):
    """Returns a tile-kernel closure over the compile-time span values."""
    ntile = nslot // 2
    slot_batch = []
    slot_seg = []
    for sg in segs:
        for k in range(sg["nslots"]):
            slot_batch.append(sg["b"])
            slot_seg.append(sg)
    # tiles owned by each batch (for the LSE reduction) -- contiguous runs
    btiles = {b: [t for t in range(ntile)
                  if slot_batch[2 * t] == b] for b in range(B)}
    for b in range(B):
        ts_ = btiles[b]
        assert all(slot_batch[2 * t] == slot_batch[2 * t + 1] for t in ts_)

    HW = 768           # h tile width: 2 slots x SLOTW
    SLOTW = 384        # per-slot region inside an h tile (252 real + pad)
    BAND = 10          # dynamic ambiguity-band width (covers 8 cores + parity)

    @with_exitstack
    def kern(ctx: ExitStack, tc: tile.TileContext, outs, ins):
        nc = tc.nc
        w1iT = ins["w1iT"]      # [768, 770] bf16 (DRAM)
        w1jT = ins["w1jT"]      # [768, 770] bf16
        w1c = ins["w1c"]        # [770] f32
        b1 = ins["b1"]          # [770] f32
        w2T6 = ins["w2T6"]      # [768, 36] bf16  (chunks 0..5 of W2^T)
        w2Tt = ins["w2Tt"]      # [3, 36] bf16    (rows 768,769 of W2^T, b2)
        vecsT = ins["vecsT"]    # [768, 504] bf16  (both batches, all j)
        myvT = ins["myvT"]      # [768, nslot] bf16 (per-core slot rows)
        maskb = ins["maskb"]    # [1, nslot*384] bf16: the m row per slot
        maskrep = ins["maskrep"]  # [nslot*252] fp16 mask per slot-column
        stepm = ins["stepm"]    # [128, 12*nin] bf16: per-core w1c window steps
        cnt2 = ins["cnt2"]      # [36, 2] f32: -252 * (padding slots of b)
        e2f = ins["e2f"]        # [128, 2] f32: col b = 1 if this core owns row s_b
        outd = outs["out"]      # [36, nslot*252] fp16
        lseo = outs["lse"]      # [36, 2] f32 (debug: total sums)

        nin = [sg["nslots"] for sg in segs if sg["kind"] == "in"]
        nin_tot = sum(nin)

        fp = ctx.enter_context(tc.tile_pool(name="fp", bufs=1))
        prep_ps = ctx.enter_context(tc.tile_pool(name="prep_ps", bufs=1, space="PSUM"))
        main_ps = ctx.enter_context(
            tc.tile_pool(name="main_ps", bufs=3 if COLPACK else 4, space="PSUM"))
        hp = [ctx.enter_context(tc.tile_pool(name=f"h{c}", bufs=4)) for c in range(7)]
        mgp = ctx.enter_context(tc.tile_pool(name="mgp", bufs=3))
        esp = ctx.enter_context(tc.tile_pool(name="esp", bufs=2))
        dram = ctx.enter_context(tc.tile_pool(name="dram", bufs=2, space="DRAM"))

        # ---- persistent SBUF ----
        s_w1i = [fp.tile([128, MLP], BF16, tag=f"w1i{k}", name=f"w1i{k}") for k in range(KC)]
        s_w1j = [fp.tile([128, MLP], BF16, tag=f"w1j{k}", name=f"w1j{k}") for k in range(KC)]
        s_vT = [fp.tile([128, 2 * L], BF16, tag=f"vT{k}", name=f"vT{k}") for k in range(KC)]
        s_myv = [fp.tile([128, nslot], BF16, tag=f"myv{k}", name=f"myv{k}") for k in range(KC)]
        s_w2 = fp.tile([128, 6 * NLAB], BF16)      # W2T chunks 0..5 side by side
        s_w2t = fp.tile([3, NLAB], BF16)           # [W2T768, W2T769, b2]
        s_w1c = fp.tile([128, 7], FP32)            # w1c per-chunk columns
        s_b1 = fp.tile([128, 7], FP32)
        s_b1w = fp.tile([128, 7], FP32)
        s_e2f = fp.tile([128, 2], FP32)
        CS = [128] * KC + [2]                      # chunk sizes of 770
        # Bj0 = Aj + b1; BjE = Aj + b1 + w1c*[j<=e]; bf16, per batch padded 512
        s_bj0 = [fp.tile([128, B * 512], BF16, tag=f"bj0_{c}", name=f"bj0_{c}") for c in range(7)]
        s_bje = [fp.tile([128, B * 512], BF16, tag=f"bje_{c}", name=f"bje_{c}") for c in range(7)]
        s_ai = [fp.tile([128, nslot], FP32, tag=f"ai{c}", name=f"ai{c}") for c in range(7)]
        s_aiE2 = [fp.tile([128, B], FP32, tag=f"aiE2{c}", name=f"aiE2{c}") for c in range(7)]
        s_out = fp.tile([NLAB, nslot * L], FP16)
        s_mrep = fp.tile([NLAB, nslot * L], FP16)
        s_stepm = fp.tile([128, 12 * nin_tot], BF16)
        s_band = [fp.tile([128, 12 * nin_tot], BF16, tag=f"bnd{c}",
                          name=f"bnd{c}") for c in range(7)]
        s_acc = fp.tile([NLAB, ntile], FP32)
        s_sums = fp.tile([NLAB, B], FP32)
        s_cnt2 = fp.tile([NLAB, B], FP32)
        s_gath = fp.tile([NLAB, NC * B], FP32)
        s_nlse = fp.tile([NLAB, B], FP32)

        # ---- load constants ----
        dmae = [nc.sync, nc.gpsimd]
        for k in range(KC):
            dmae[k % 2].dma_start(out=s_w1i[k], in_=w1iT[128 * k:128 * (k + 1), :])
            dmae[(k + 1) % 2].dma_start(out=s_w1j[k],
                                        in_=w1jT[128 * k:128 * (k + 1), :])
            dmae[k % 2].dma_start(out=s_vT[k],
                                        in_=vecsT[128 * k:128 * (k + 1), :])
            dmae[(k + 1) % 2].dma_start(out=s_myv[k],
                                        in_=myvT[128 * k:128 * (k + 1), :])
            dmae[k % 2].dma_start(out=s_w2[:, NLAB * k:NLAB * (k + 1)],
                                        in_=w2T6[128 * k:128 * (k + 1), :])
        nc.sync.dma_start(out=s_w2t, in_=w2Tt)
        # w1c/b1 [770] -> [128, 7] (partition p, col c) = v[128*c + p]
        nc.vector.memset(s_w1c, 0.0)
        nc.vector.memset(s_b1, 0.0)
        w1c2 = w1c[0:HID].rearrange("(c p) -> p c", p=128)
        nc.sync.dma_start(out=s_w1c[:, 0:KC], in_=w1c2)
        nc.sync.dma_start(out=s_w1c[0:2, KC:7], in_=w1c[HID:MLP].rearrange("(p o) -> p o", o=1))
        b12 = b1[0:HID].rearrange("(c p) -> p c", p=128)
        nc.sync.dma_start(out=s_b1[:, 0:KC], in_=b12)
        nc.sync.dma_start(out=s_b1[0:2, KC:7], in_=b1[HID:MLP].rearrange("(p o) -> p o", o=1))
        nc.sync.dma_start(out=s_e2f, in_=e2f)
        nc.vector.tensor_tensor(out=s_b1w, in0=s_b1, in1=s_w1c, op=ALU.add)
        nc.sync.dma_start(out=s_cnt2, in_=cnt2)
        # mask replicated over the 36 label partitions, loaded once
        nc.sync.dma_start(
            out=s_mrep,
            in_=bass.AP(tensor=maskrep.tensor, offset=maskrep.offset,
                        ap=[[0, NLAB], [1, nslot * L]]))
        nc.sync.dma_start(out=s_stepm, in_=stepm)

        # ---- prep: AjT -> Bj0/BjE, AiT ----
        for c in range(7):
            cs = CS[c]
            mlo = 128 * c
            # Bj (= Aj + b1) for all 504 (b, j) columns
            psA = prep_ps.tile([128, 2 * L], FP32, tag="psA", name=f"psA{c}")
            for k in range(KC):
                nc.tensor.matmul(psA[:cs, :], s_w1j[k][:, mlo:mlo + cs], s_vT[k],
                                 start=(k == 0), stop=(k == KC - 1))
            bje = s_bje[c]
            for b in range(B):
                e = spans[b][1]
                nc.vector.tensor_scalar(
                    out=s_bj0[c][:cs, 512 * b:512 * b + L],
                    in0=psA[:cs, L * b:L * (b + 1)],
                    scalar1=s_b1[:cs, c:c + 1], scalar2=None, op0=ALU.add)
                nc.gpsimd.memset(s_bj0[c][:cs, 512 * b + L:512 * (b + 1)], 0.0)
                nc.vector.tensor_scalar(
                    out=bje[:cs, 512 * b:512 * b + e + 1],
                    in0=psA[:cs, L * b:L * b + e + 1],
                    scalar1=s_b1w[:cs, c:c + 1], scalar2=None, op0=ALU.add)
                if e + 1 < L:
                    nc.vector.tensor_scalar(
                        out=bje[:cs, 512 * b + e + 1:512 * b + L],
                        in0=psA[:cs, L * b + e + 1:L * (b + 1)],
                        scalar1=s_b1[:cs, c:c + 1], scalar2=None, op0=ALU.add)
                nc.gpsimd.memset(bje[:cs, 512 * b + L:512 * (b + 1)], 0.0)
            # Band source: per in-span slot q the 12 cols [B0_q, B0_q+12) of
            # Bj0 plus w1c * stepm (the per-core window-entry pattern).
            # Gather the overlapping windows through a DRAM bounce.
            dbj = dram.tile([128, B * 512], BF16, tag="dbj", name=f"dbj{c}")
            nc.sync.dma_start(out=dbj[0:cs, :], in_=s_bj0[c][:cs, :])
            bg = mgp.tile([128, 12 * nin_tot], BF16, tag="bg", name=f"bg{c}")
            q0 = 0
            for bi, sg in enumerate(sg for sg in segs if sg["kind"] == "in"):
                b, s_b, nin_b = sg["b"], sg["s"], sg["nslots"]
                src = dbj[0:cs, 512 * b + s_b:512 * b + s_b + 1]
                nc.sync.dma_start(
                    out=bg[:cs, 12 * q0:12 * (q0 + nin_b)],
                    in_=bass.AP(tensor=src.tensor, offset=src.offset,
                                ap=[[B * 512, cs], [8, nin_b], [1, 12]]))
                q0 += nin_b
            t1 = mgp.tile([128, 12 * nin_tot], BF16, tag="t1", name=f"t1{c}")
            nc.vector.tensor_scalar(out=t1[:cs, :], in0=s_stepm[:cs, :],
                                    scalar1=s_w1c[:cs, c:c + 1], scalar2=None,
                                    op0=ALU.mult)
            nc.vector.tensor_tensor(out=s_band[c][:cs, :], in0=t1[:cs, :],
                                    in1=bg[:cs, :], op=ALU.add)
            # AiT for my slots
            psI = prep_ps.tile([128, nslot], FP32, tag="psI", name=f"psI{c}")
            for k in range(KC):
                nc.tensor.matmul(psI[:cs, :], s_w1i[k][:, mlo:mlo + cs], s_myv[k],
                                 start=(k == 0), stop=(k == KC - 1))
            nc.vector.tensor_copy(out=s_ai[c][:cs, :], in_=psI[:cs, :])
            # E2 bias: Ai[slot_e2] + w1c * own_flag  (per batch)
            for b in range(B):
                p_e2 = next(sg for sg in segs
                            if sg["kind"] == "in" and sg["b"] == b)["start"]
                nc.vector.tensor_tensor(
                    out=s_aiE2[c][:cs, b:b + 1],
                    in0=s_w1c[:cs, c:c + 1], in1=s_e2f[:cs, b:b + 1],
                    op=ALU.mult)
                nc.vector.tensor_tensor(
                    out=s_aiE2[c][:cs, b:b + 1],
                    in0=s_aiE2[c][:cs, b:b + 1], in1=s_ai[c][:cs, p_e2:p_e2 + 1],
                    op=ALU.add)

        def ts_relu(eng, out, in0, sc):
            if eng is nc.scalar:
                nc.scalar.activation(out, in0, AF.Relu, bias=sc, scale=1.0)
            else:
                eng.tensor_scalar(out=out, in0=in0, scalar1=sc, scalar2=0.0,
                                  op0=ALU.add, op1=ALU.max)

        # estimated per-engine cost (ns) of an N-col relu op, for balancing
        def op_cost(eng, n):
            if eng is nc.vector:
                return (160 + n / 4) / 0.96
            if eng is nc.scalar:
                return (96 + n) / 0.96
            return 95 + n * 0.714 / 0.6

        # ---- main loop over 2-slot tiles ----
        for t in range(ntile):
            hts = []
            for c in range(7):
                cs = CS[c] if c < 6 else 3
                ht = hp[c].tile([cs, HW], BF16, tag=f"ht{c}", name=f"ht{c}_{t}")
                hts.append(ht)
            nc.sync.dma_start(out=hts[6][2:3, :], in_=maskb[:, HW * t:HW * (t + 1)])
            # collect this tile's relu ops, then greedily balance engines.
            # fixed per-tile loads: DVE runs the TT merge+mask chain, ACT the
            # hi-copy and exp, GpSimd the mask TT.
            load = {id(nc.vector): 660.0, id(nc.scalar): 680.0 + 960.0,
                    id(nc.gpsimd): 1210.0}
            engs = [nc.vector, nc.scalar, nc.gpsimd]
            ops = []   # (ncols, out, in0, scalar, order_group)
            for c in range(7):
                cs = CS[c]
                ht = hts[c]
                for sl in range(2):
                    p = 2 * t + sl
                    b = slot_batch[p]
                    sg = slot_seg[p]
                    base = SLOTW * sl
                    if sg["kind"] != "in":
                        ops.append((256, ht[:cs, base:base + 256],
                                    s_bj0[c][:cs, 512 * b:512 * b + 256],
                                    s_ai[c][:cs, p:p + 1], 0))
                        continue
                    s, e = sg["s"], sg["e"]
                    kk = p - sg["start"]
                    B0 = s + 8 * kk
                    bw = 8 + (B0 & 1)
                    SBe = B0 + bw                    # even static window start
                    Aw = min(SBe + (SBe & 1), 256)   # segA prefix end (even)
                    # segA: [0, Aw) from Bj0 (covers j < i for every core)
                    ops.append((Aw, ht[:cs, base:base + Aw],
                                s_bj0[c][:cs, 512 * b:512 * b + Aw],
                                s_ai[c][:cs, p:p + 1], 0))
                    # band: [B0, B0+bw) from the precomputed per-core source
                    ops.append((bw, ht[:cs, base + B0:base + SBe],
                                s_band[c][:cs, 12 * p:12 * p + bw],
                                s_ai[c][:cs, p:p + 1], 1))
                    # static suffix [SBe, 256) from BjE (== Bj0 past e)
                    if SBe < 256:
                        ops.append((256 - SBe, ht[:cs, base + SBe:base + 256],
                                    s_bje[c][:cs, 512 * b + SBe:512 * b + 256],
                                    s_ai[c][:cs, p:p + 1], 2))
            # issue in order-group order so overwrites are sequenced, engine
            # chosen greedily by estimated completion time
            ops.sort(key=lambda o: o[4])
            for (ncol, o_ap, i_ap, sc, grp) in ops:
                eng = min(engs, key=lambda en: load[id(en)] + op_cost(en, ncol))
                load[id(eng)] += op_cost(eng, ncol)
                ts_relu(eng, o_ap, i_ap, sc)
            # E2 pixel: slot holding row s_b gets +2*w1c at j=e
            for c in range(7):
                cs = CS[c]
                for sl in range(2):
                    p = 2 * t + sl
                    sg = slot_seg[p]
                    if sg["kind"] == "in" and p == sg["start"]:
                        b, e = sg["b"], sg["e"]
                        base = SLOTW * sl
                        ts_relu(nc.vector, hts[c][:cs, base + e:base + e + 1],
                                s_bje[c][:cs, 512 * b + e:512 * b + e + 1],
                                s_aiE2[c][:cs, b:b + 1])

            # matmul: 6 chunks col-packed into two PE column groups + tail
            rhs6 = [hts[c][:, :].rearrange("p (s w) -> p s w", w=SLOTW)
                    [:, :, 0:L] for c in range(6)]
            rhs_t = hts[6][:, :].rearrange("p (s w) -> p s w", w=SLOTW)[:, :, 0:L]
            if COLPACK:
                ps = main_ps.tile([128, 2 * 512], FP32, tag="ps", name=f"ps{t}")
                lo = ps[0:NLAB, 0:2 * L]
                hi = ps[64:64 + NLAB, 512:512 + 2 * L]
                for c in range(6):
                    col = 64 * (c % 2)
                    nc.tensor.matmul(lo if col == 0 else hi,
                                     s_w2[:, NLAB * c:NLAB * (c + 1)], rhs6[c],
                                     start=(c < 2), stop=(c == 5),
                                     tile_position=(0, col))
                nc.tensor.matmul(lo, s_w2t, rhs_t, start=False, stop=True)
                # merge the two half-sums -> fp16 logits (engines can read at
                # most one PSUM operand: stage the hi half through SBUF on ACT)
                hi_sb = mgp.tile([NLAB, 2 * L], FP32, tag="hisb", name=f"hisb{t}")
                nc.scalar.activation(hi_sb, hi, AF.Copy)
                mg = mgp.tile([NLAB, 2 * L], FP16, tag="mg", name=f"mg{t}")
                nc.vector.tensor_tensor(out=mg, in0=lo, in1=hi_sb, op=ALU.add)
            else:
                ps = main_ps.tile([NLAB, 2 * L], FP32, tag="ps", name=f"ps{t}")
                for c in range(6):
                    nc.tensor.matmul(ps, s_w2[:, NLAB * c:NLAB * (c + 1)], rhs6[c],
                                     start=(c == 0), stop=False)
                nc.tensor.matmul(ps, s_w2t, rhs_t, start=False, stop=True)
                mg = ps

            # mask -> exact zeros for invalid pairs (fp16), then exp+sum;
            # invalid pairs contribute exp(0)=1 exactly like the reference
            seg = s_out[:, 2 * L * t:2 * L * (t + 1)]
            mask_eng = nc.gpsimd if COLPACK else nc.vector
            mask_eng.tensor_tensor(out=seg, in0=mg,
                                   in1=s_mrep[:, 2 * L * t:2 * L * (t + 1)],
                                   op=ALU.mult)
            esc = esp.tile([NLAB, 2 * L], FP16, tag="esc", name=f"esc{t}")
            nc.scalar.activation(esc, seg, AF.Exp, accum_out=s_acc[:, t:t + 1])

        # ---- LSE: per-batch sums + AllGather + local tree + log ----
        for b in range(B):
            rs = btiles[b]
            runs = []
            st = rs[0]
            for a, bb in zip(rs, rs[1:] + [None]):
                if bb != a + 1:
                    runs.append((st, a))
                    st = bb
            t0, t1 = runs[0]
            nc.vector.tensor_reduce(out=s_sums[:, b:b + 1],
                                    in_=s_acc[:, t0:t1 + 1],
                                    axis=mybir.AxisListType.X, op=ALU.add)
            for (u0, u1) in runs[1:]:
                tmp = fp.tile([NLAB, 1], FP32, tag=f"tr{b}", name=f"tr{b}_{u0}")
                nc.vector.tensor_reduce(out=tmp, in_=s_acc[:, u0:u1 + 1],
                                        axis=mybir.AxisListType.X, op=ALU.add)
                nc.vector.tensor_tensor(out=s_sums[:, b:b + 1],
                                        in0=s_sums[:, b:b + 1], in1=tmp,
                                        op=ALU.add)
        # correct for padding slots (each contributed 252 * exp(0))
        nc.vector.tensor_tensor(out=s_sums, in0=s_sums, in1=s_cnt2, op=ALU.add)
        cc_in = dram.tile([NLAB, B], FP32, name="cc_in")
        cc_out = nc.dram_tensor("cc_out", [NC, NLAB, B], FP32, kind="Internal",
                                addr_space="Shared").ap()
        nc.sync.dma_start(out=cc_in, in_=s_sums)
        nc.gpsimd.collective_compute(
            "AllGather", ALU.bypass, replica_groups=[list(range(NC))],
            ins=[cc_in[:]], outs=[cc_out], cc_dim="Partition")
        nc.sync.dma_start(out=s_gath.rearrange("k (r c) -> k r c", c=B),
                          in_=cc_out.rearrange("r k c -> k r c"))
        # tree-reduce the 8 rank contributions: cols (r*B + c)
        nc.vector.tensor_tensor(out=s_gath[:, 0:8], in0=s_gath[:, 0:8],
                                in1=s_gath[:, 8:16], op=ALU.add)
        nc.vector.tensor_tensor(out=s_gath[:, 0:4], in0=s_gath[:, 0:4],
                                in1=s_gath[:, 4:8], op=ALU.add)
        nc.vector.tensor_tensor(out=s_gath[:, 0:2], in0=s_gath[:, 0:2],
                                in1=s_gath[:, 2:4], op=ALU.add)
        nc.sync.dma_start(out=lseo, in_=s_gath[:, 0:2])
        nc.scalar.activation(s_nlse, s_gath[:, 0:2], AF.Ln)
        nc.vector.tensor_scalar(out=s_nlse, in0=s_nlse, scalar1=-1.0,
                                scalar2=None, op0=ALU.mult)

        # ---- final: out - LSE over the 4 contiguous batch runs, store ----
        allruns = []
        for b in range(B):
            rs = btiles[b]
            st = rs[0]
            for a, bb in zip(rs, rs[1:] + [None]):
                if bb != a + 1:
                    allruns.append((b, st, a))
                    st = bb
        for i, (b, u0, u1) in enumerate(allruns):
            seg = s_out[:, 2 * L * u0:2 * L * (u1 + 1)]
            nc.vector.tensor_scalar(out=seg, in0=seg,
                                    scalar1=s_nlse[:, b:b + 1], scalar2=None,
                                    op0=ALU.add)
            dmae[i % 2].dma_start(out=outd[:, 2 * L * u0:2 * L * (u1 + 1)],
                                  in_=seg)

    return kern


def kernel(**inputs) -> np.ndarray:
    hidden = np.asarray(inputs["hidden"], dtype=np.float32)
    pred_spans = np.asarray(inputs["pred_spans"]).astype(np.int64)
    span_mask = np.asarray(inputs["span_mask"]).astype(np.int32)
    W1 = np.asarray(inputs["W1"], dtype=np.float32)
    b1 = np.asarray(inputs["b1"], dtype=np.float32)
    W2 = np.asarray(inputs["W2"], dtype=np.float32)
    b2 = np.asarray(inputs["b2"], dtype=np.float32)

    spans = [(int(pred_spans[b, 0]), int(pred_spans[b, 1])) for b in range(B)]
    segs, nslot = plan_slots(spans)
    ntile = nslot // 2

    vecs = hidden[:, 1:L + 1, :]                       # [B, L, 768]
    vecsT = np.concatenate([vecs[0].T, vecs[1].T], axis=1)   # [768, 504]
    W1T = W1.T                                          # [1537, 770]
    w1iT = np.ascontiguousarray(W1T[0:HID]).astype(np.float32)
    w1jT = np.ascontiguousarray(W1T[HID:2 * HID]).astype(np.float32)
    w1c = np.ascontiguousarray(W1T[2 * HID]).astype(np.float32)
    W2T = np.ascontiguousarray(W2.T)                    # [770, 36]
    w2T6 = W2T[0:HID]
    w2Tt = np.stack([W2T[768], W2T[769], b2], axis=0)   # [3, 36]

    maskf_full = span_mask.astype(np.float32).clip(0, 1)
    nin_tot = sum(sg["nslots"] for sg in segs if sg["kind"] == "in")

    in_maps = []
    slot_maps = []
    for c in range(NC):
        sm = slot_map_for_core(segs, nslot, c)
        slot_maps.append(sm)
        myv = np.zeros((HID, nslot), np.float32)
        maskrep = np.zeros(nslot * L, np.float32)
        npad = [0, 0]
        for p, ent in enumerate(sm):
            if ent is None:
                npad[next(sg["b"] for sg in segs
                          if sg["start"] <= p < sg["start"] + sg["nslots"])] += 1
                continue
            b, r = ent
            myv[:, p] = vecs[b, r]
            maskrep[L * p:L * (p + 1)] = maskf_full[r]
        maskb = np.zeros((1, nslot * 384), np.float32)
        for p, ent in enumerate(sm):
            if ent is not None:
                o = 384 * p
                maskb[0, o:o + L] = maskf_full[ent[1]]
        cnt2 = np.zeros((NLAB, B), np.float32)
        for b in range(B):
            cnt2[:, b] = -float(L * npad[b])
        stepm = np.zeros((128, 12 * nin_tot), np.float32)
        for sg in segs:
            if sg["kind"] != "in":
                continue
            s, e = sg["s"], sg["e"]
            for kk in range(sg["nslots"]):
                q = sg["start"] + kk
                B0 = s + 8 * kk
                for d in range(12):
                    if d >= c and B0 + d <= e:
                        stepm[:, 12 * q + d] = 1.0
        e2f = np.zeros((128, 2), np.float32)
        for b in range(B):
            if c == 0:
                e2f[:, b] = 1.0      # row s_b lives on core 0 (slot seg start)
        in_maps.append({
            "w1iT": w1iT, "w1jT": w1jT, "w1c": w1c, "b1": b1,
            "w2T6": w2T6, "w2Tt": w2Tt,
            "vecsT": vecsT, "myvT": myv,
            "maskb": maskb, "maskrep": maskrep, "stepm": stepm, "cnt2": cnt2,
            "e2f": e2f,
        })

    # ---- build program ----
    nc = bacc.Bacc("TRN2", target_bir_lowering=False, debug=False,
                   enable_asserts=False, num_devices=NC)

    def mk(name, arr, dt):
        return nc.dram_tensor(name, list(arr.shape), dt, kind="ExternalInput").ap()

    ml_bf = lambda n, a: mk(n, a, BF16)
    ml_f16 = lambda n, a: mk(n, a, FP16)
    ml_f32 = lambda n, a: mk(n, a, FP32)
    ex = in_maps[0]
    ins_aps = {
        "w1iT": ml_bf("w1iT", ex["w1iT"]), "w1jT": ml_bf("w1jT", ex["w1jT"]),
        "w1c": ml_f32("w1c", ex["w1c"]), "b1": ml_f32("b1", ex["b1"]),
        "w2T6": ml_bf("w2T6", ex["w2T6"]), "w2Tt": ml_bf("w2Tt", ex["w2Tt"]),
        "vecsT": ml_bf("vecsT", ex["vecsT"]), "myvT": ml_bf("myvT", ex["myvT"]),
        "maskb": ml_bf("maskb", ex["maskb"]),
        "maskrep": ml_f16("maskrep", ex["maskrep"]),
        "stepm": ml_bf("stepm", ex["stepm"]),
        "cnt2": ml_f32("cnt2", ex["cnt2"]), "e2f": ml_f32("e2f", ex["e2f"]),
    }
    outs_aps = {
        "out": nc.dram_tensor("out", [NLAB, nslot * L], FP16,
                              kind="ExternalOutput").ap(),
        "lse": nc.dram_tensor("lse", [NLAB, B], FP32,
                              kind="ExternalOutput").ap(),
    }

    kern = build_kernel(spans, segs, nslot)
    with tile.TileContext(nc) as t:
        kern(t, outs_aps, ins_aps)
    nc.compile()

    def cast_maps(m):
        out = {}
        for k, v in m.items():
            dt = ins_aps[k].dtype
            out[k] = v.astype(mybir.dt.np(dt))
        return out

    in_maps_c = [cast_maps(m) for m in in_maps]

    if os.environ.get("BK_BUILD_ONLY"):
        print("BUILD OK")
        return np.zeros((B, NLAB, L * L), np.float32)

    if os.environ.get("BK_SIM"):
        from concourse.bass_interp import MultiCoreSim

        sim = MultiCoreSim(nc, num_cores=NC, require_finite=False,
                           require_nnan=False)
        for c, cs in sim.cores.items():
            for name, arr in in_maps_c[c].items():
                cs.tensor(name)[:] = arr
            if nc.partition_id_tensor is not None:
                cs.tensor(nc.partition_id_tensor.name)[:] = np.array(
                    [[c]], dtype=np.uint32)
        sim.simulate(check_with_hw=False)

        class _R:
            results = [{"out": np.asarray(sim.cores[c].tensor("out")),
                        "lse": np.asarray(sim.cores[c].tensor("lse"))}
                       for c in range(NC)]
        res = _R()
    else:
        trace = bool(int(os.environ.get("BK_TRACE", "0")))
        res = run_bass_kernel_spmd(nc, in_maps_c, core_ids=list(range(NC)),
                                   trace=trace)
        if trace and res.exec_time_ns is not None:
            print(f"HW exec time: {res.exec_time_ns} ns")

    # ---- unshard ----
    out_full = np.zeros((B, NLAB, L * L), np.float32)
    for c in range(NC):
        oc = np.asarray(res.results[c]["out"]).astype(np.float32)
        for p, ent in enumerate(slot_maps[c]):
            if ent is None:
                continue
            b, r = ent
            out_full[b, :, L * r:L * (r + 1)] = oc[:, L * p:L * (p + 1)]
    return out_full


# revision 21
# speedup vs baseline: 2.0892x; 2.0892x over previous
"""Trainium2 Bass kernel for the BERT span-pair classifier problem.

Computes, for B=2 batches over a 252x252 span-pair grid:
    h    = relu(Ai[i] + Aj[j] + ind(i,j)*w1c + b1)        # [770] per pair
    out  = h @ W2.T + b2                                   # [36]  per pair
    out  = where(span_mask >= 1, out, 0)
    res  = log_softmax(out over the 63504 pairs)           # per (batch, label)
    return res transposed to [B, 36, L*L]

Strategy (8 NeuronCores, SPMD single program):
  - The 504 (batch, row-i) rows are distributed over 8 cores, grouped into 4
    slot segments (in-span b0, in-span b1, off-span b0, off-span b1) so that
    every core runs an identical instruction template per slot.  In-span
    segments are assigned round-robin (row = s_b + 8*k + core).
  - h is produced in [hid-chunk(128) x j] layout, bf16 end to end so the DVE
    tensor_scalar ops hit the packed 4x mode.  Per in-span slot the w1c
    window [i, e] is written as a static suffix [B0+8(+parity), e+1] from BjE
    plus a single 9-wide dynamic ACT op covering the per-core ambiguity band.
  - The 770->36 matmul streams h against stationary W2^T chunks, two chunks
    at a time in separate PE column groups (tile_position col packing); the
    two PSUM half-sums are merged and masked into fp16 s_out.
  - exp runs on the *masked* logits so invalid pairs contribute exp(0)=1
    exactly as the reference does; padding slots are corrected by a host
    constant.  The cross-core reduction is a tiny AllGather + local tree.
  - The final (out - lse) uses 4 big fp16 4x tensor_scalar ops + 4 DMAs.
"""

import math
import os
from contextlib import ExitStack

import numpy as np

import concourse.bass as bass
import concourse.bacc as bacc
import concourse.tile as tile
from concourse import mybir
from concourse._compat import with_exitstack
from concourse.bass_utils import run_bass_kernel_spmd

L = 252
HID = 768
MLP = 770
NLAB = 36
B = 2
NC = 8
KC = 6           # full 128-row hid chunks (6*128 = 768)

FP32 = mybir.dt.float32
FP16 = mybir.dt.float16
BF16 = mybir.dt.bfloat16
AF = mybir.ActivationFunctionType
ALU = mybir.AluOpType

COLPACK = True   # two W2 chunks in separate PE column groups


def _ceil2(x):
    return x + (x & 1)


def plan_slots(spans):
    """Compute the slot layout shared by host + device.

    Returns dict with per-segment (kind, batch, start_slot, nslots, rows).
    In-span segments: core c, slot p holds global row  s_b + 8*(p-start) + c.
    Off-span segments: arbitrary round-robin over the listed rows.
    """
    segs = []
    slot = 0
    for b in range(B):
        s, e = spans[b]
        n = e - s + 1
        nsl = _ceil2(math.ceil(n / NC))
        segs.append(dict(kind="in", b=b, start=slot, nslots=nsl, s=s, e=e, count=n))
        slot += nsl
    for b in range(B):
        s, e = spans[b]
        rows = [r for r in range(L) if r < s or r > e]
        nsl = _ceil2(math.ceil(len(rows) / NC))
        segs.append(dict(kind="off", b=b, start=slot, nslots=nsl, rows=rows,
                         count=len(rows)))
        slot += nsl
    nslot = slot
    assert nslot % 2 == 0
    return segs, nslot


def slot_map_for_core(segs, nslot, c):
    """-> list over slots of (batch, global_row) or None for padding."""
    m = [None] * nslot
    for sg in segs:
        for k in range(sg["nslots"]):
            idx = NC * k + c
            p = sg["start"] + k
            if idx < sg["count"]:
                if sg["kind"] == "in":
                    m[p] = (sg["b"], sg["s"] + idx)
                else:
                    m[p] = (sg["b"], sg["rows"][idx])
    return m


def build_kernel(spans, segs, nslot):
    """Returns a tile-kernel closure over the compile-time span values."""
    ntile = nslot // 2
    slot_batch = []
    slot_seg = []
    for sg in segs:
        for k in range(sg["nslots"]):
            slot_batch.append(sg["b"])
            slot_seg.append(sg)
    # tiles owned by each batch (for the LSE reduction) -- contiguous runs
    btiles = {b: [t for t in range(ntile)
                  if slot_batch[2 * t] == b] for b in range(B)}
    for b in range(B):
        ts_ = btiles[b]
        assert all(slot_batch[2 * t] == slot_batch[2 * t + 1] for t in ts_)

    HW = 768           # h tile width: 2 slots x SLOTW
    SLOTW = 384        # per-slot region inside an h tile (252 real + pad)
    BAND = 10          # dynamic ambiguity-band width (covers 8 cores + parity)

    @with_exitstack
    def kern(ctx: ExitStack, tc: tile.TileContext, outs, ins):
        nc = tc.nc
        w1iT = ins["w1iT"]      # [768, 770] bf16 (DRAM)
        w1jT = ins["w1jT"]      # [768, 770] bf16
        w1c = ins["w1c"]        # [770] f32
        b1 = ins["b1"]          # [770] f32
        w2T6 = ins["w2T6"]      # [768, 36] bf16  (chunks 0..5 of W2^T)
        w2Tt = ins["w2Tt"]      # [3, 36] bf16    (rows 768,769 of W2^T, b2)
        vecsT = ins["vecsT"]    # [768, 504] bf16  (both batches, all j)
        myvT = ins["myvT"]      # [768, nslot] bf16 (per-core slot rows)
        maskb = ins["maskb"]    # [1, nslot*384] bf16: the m row per slot
        maskrep = ins["maskrep"]  # [nslot*252] fp16 mask per slot-column
        stepm = ins["stepm"]    # [128, 12*nin] bf16: per-core w1c window steps
        cnt2 = ins["cnt2"]      # [36, 2] f32: -252 * (padding slots of b)
        e2f = ins["e2f"]        # [128, 2] f32: col b = 1 if this core owns row s_b
        outd = outs["out"]      # [36, nslot*252] fp16
        lseo = outs["lse"]      # [36, 2] f32 (debug: total sums)

        nin = [sg["nslots"] for sg in segs if sg["kind"] == "in"]
        nin_tot = sum(nin)

        fp = ctx.enter_context(tc.tile_pool(name="fp", bufs=1))
        prep_ps = ctx.enter_context(tc.tile_pool(name="prep_ps", bufs=1, space="PSUM"))
        main_ps = ctx.enter_context(
            tc.tile_pool(name="main_ps", bufs=3 if COLPACK else 4, space="PSUM"))
        hp = [ctx.enter_context(tc.tile_pool(name=f"h{c}", bufs=4)) for c in range(7)]
        mgp = ctx.enter_context(tc.tile_pool(name="mgp", bufs=3))
        esp = ctx.enter_context(tc.tile_pool(name="esp", bufs=2))
        dram = ctx.enter_context(tc.tile_pool(name="dram", bufs=2, space="DRAM"))

        # ---- persistent SBUF ----
        s_w1i = [fp.tile([128, MLP], BF16, tag=f"w1i{k}", name=f"w1i{k}") for k in range(KC)]
        s_w1j = [fp.tile([128, MLP], BF16, tag=f"w1j{k}", name=f"w1j{k}") for k in range(KC)]
        s_vT = [fp.tile([128, 2 * L], BF16, tag=f"vT{k}", name=f"vT{k}") for k in range(KC)]
        s_myv = [fp.tile([128, nslot], BF16, tag=f"myv{k}", name=f"myv{k}") for k in range(KC)]
        s_w2 = fp.tile([128, 6 * NLAB], BF16)      # W2T chunks 0..5 side by side
        s_w2t = fp.tile([3, NLAB], BF16)           # [W2T768, W2T769, b2]
        s_w1c = fp.tile([128, 7], FP32)            # w1c per-chunk columns
        s_b1 = fp.tile([128, 7], FP32)
        s_b1w = fp.tile([128, 7], FP32)
        s_e2f = fp.tile([128, 2], FP32)
        CS = [128] * KC + [2]                      # chunk sizes of 770
        # Bj0 = Aj + b1; BjE = Aj + b1 + w1c*[j<=e]; bf16, per batch padded 512
        s_bj0 = [fp.tile([128, B * 512], BF16, tag=f"bj0_{c}", name=f"bj0_{c}") for c in range(7)]
        s_bje = [fp.tile([128, B * 512], BF16, tag=f"bje_{c}", name=f"bje_{c}") for c in range(7)]
        s_ai = [fp.tile([128, nslot], FP32, tag=f"ai{c}", name=f"ai{c}") for c in range(7)]
        s_aiE2 = [fp.tile([128, B], FP32, tag=f"aiE2{c}", name=f"aiE2{c}") for c in range(7)]
        s_out = fp.tile([NLAB, nslot * L], FP16)
        s_mrep = fp.tile([NLAB, nslot * L], FP16)
        s_stepm = fp.tile([128, 12 * nin_tot], BF16)
        s_band = [fp.tile([128, 12 * nin_tot], BF16, tag=f"bnd{c}",
                          name=f"bnd{c}") for c in range(7)]
        s_acc = fp.tile([NLAB, ntile], FP32)
        s_sums = fp.tile([NLAB, B], FP32)
        s_cnt2 = fp.tile([NLAB, B], FP32)
        s_gath = fp.tile([NLAB, NC * B], FP32)
        s_nlse = fp.tile([NLAB, B], FP32)

        # ---- load constants ----
        dmae = [nc.sync, nc.gpsimd]
        for k in range(KC):
            dmae[k % 2].dma_start(out=s_w1i[k], in_=w1iT[128 * k:128 * (k + 1), :])
            dmae[(k + 1) % 2].dma_start(out=s_w1j[k],
                                        in_=w1jT[128 * k:128 * (k + 1), :])
            dmae[k % 2].dma_start(out=s_vT[k],
                                        in_=vecsT[128 * k:128 * (k + 1), :])
            dmae[(k + 1) % 2].dma_start(out=s_myv[k],
                                        in_=myvT[128 * k:128 * (k + 1), :])
            dmae[k % 2].dma_start(out=s_w2[:, NLAB * k:NLAB * (k + 1)],
                                        in_=w2T6[128 * k:128 * (k + 1), :])
        nc.sync.dma_start(out=s_w2t, in_=w2Tt)
        # w1c/b1 [770] -> [128, 7] (partition p, col c) = v[128*c + p]
        nc.vector.memset(s_w1c, 0.0)
        nc.vector.memset(s_b1, 0.0)
        w1c2 = w1c[0:HID].rearrange("(c p) -> p c", p=128)
        nc.sync.dma_start(out=s_w1c[:, 0:KC], in_=w1c2)
        nc.sync.dma_start(out=s_w1c[0:2, KC:7], in_=w1c[HID:MLP].rearrange("(p o) -> p o", o=1))
        b12 = b1[0:HID].rearrange("(c p) -> p c", p=128)
        nc.sync.dma_start(out=s_b1[:, 0:KC], in_=b12)
        nc.sync.dma_start(out=s_b1[0:2, KC:7], in_=b1[HID:MLP].rearrange("(p o) -> p o", o=1))
        nc.sync.dma_start(out=s_e2f, in_=e2f)
        nc.vector.tensor_tensor(out=s_b1w, in0=s_b1, in1=s_w1c, op=ALU.add)
        nc.sync.dma_start(out=s_cnt2, in_=cnt2)
        # mask replicated over the 36 label partitions, loaded once
        nc.sync.dma_start(
            out=s_mrep,
            in_=bass.AP(tensor=maskrep.tensor, offset=maskrep.offset,
                        ap=[[0, NLAB], [1, nslot * L]]))
        nc.sync.dma_start(out=s_stepm, in_=stepm)

        # ---- prep: AjT -> Bj0/BjE, AiT ----
        for c in range(7):
            cs = CS[c]
            mlo = 128 * c
            # Bj (= Aj + b1) for all 504 (b, j) columns
            psA = prep_ps.tile([128, 2 * L], FP32, tag="psA", name=f"psA{c}")
            for k in range(KC):
                nc.tensor.matmul(psA[:cs, :], s_w1j[k][:, mlo:mlo + cs], s_vT[k],
                                 start=(k == 0), stop=(k == KC - 1))
            bje = s_bje[c]
            for b in range(B):
                e = spans[b][1]
                nc.vector.tensor_scalar(
                    out=s_bj0[c][:cs, 512 * b:512 * b + L],
                    in0=psA[:cs, L * b:L * (b + 1)],
                    scalar1=s_b1[:cs, c:c + 1], scalar2=None, op0=ALU.add)
                nc.gpsimd.memset(s_bj0[c][:cs, 512 * b + L:512 * (b + 1)], 0.0)
                nc.vector.tensor_scalar(
                    out=bje[:cs, 512 * b:512 * b + e + 1],
                    in0=psA[:cs, L * b:L * b + e + 1],
                    scalar1=s_b1w[:cs, c:c + 1], scalar2=None, op0=ALU.add)
                if e + 1 < L:
                    nc.vector.tensor_scalar(
                        out=bje[:cs, 512 * b + e + 1:512 * b + L],
                        in0=psA[:cs, L * b + e + 1:L * (b + 1)],
                        scalar1=s_b1[:cs, c:c + 1], scalar2=None, op0=ALU.add)
                nc.gpsimd.memset(bje[:cs, 512 * b + L:512 * (b + 1)], 0.0)
            # Band source: per in-span slot q the 12 cols [B0_q, B0_q+12) of
            # Bj0 plus w1c * stepm (the per-core window-entry pattern).
            # Gather the overlapping windows through a DRAM bounce.
            dbj = dram.tile([128, B * 512], BF16, tag="dbj", name=f"dbj{c}")
            nc.sync.dma_start(out=dbj[0:cs, :], in_=s_bj0[c][:cs, :])
            bg = mgp.tile([128, 12 * nin_tot], BF16, tag="bg", name=f"bg{c}")
            q0 = 0
            for bi, sg in enumerate(sg for sg in segs if sg["kind"] == "in"):
                b, s_b, nin_b = sg["b"], sg["s"], sg["nslots"]
                src = dbj[0:cs, 512 * b + s_b:512 * b + s_b + 1]
                nc.sync.dma_start(
                    out=bg[:cs, 12 * q0:12 * (q0 + nin_b)],
                    in_=bass.AP(tensor=src.tensor, offset=src.offset,
                                ap=[[B * 512, cs], [8, nin_b], [1, 12]]))
                q0 += nin_b
            t1 = mgp.tile([128, 12 * nin_tot], BF16, tag="t1", name=f"t1{c}")
            nc.vector.tensor_scalar(out=t1[:cs, :], in0=s_stepm[:cs, :],
                                    scalar1=s_w1c[:cs, c:c + 1], scalar2=None,
                                    op0=ALU.mult)
            nc.vector.tensor_tensor(out=s_band[c][:cs, :], in0=t1[:cs, :],
                                    in1=bg[:cs, :], op=ALU.add)
            # AiT for my slots
            psI = prep_ps.tile([128, nslot], FP32, tag="psI", name=f"psI{c}")
            for k in range(KC):
                nc.tensor.matmul(psI[:cs, :], s_w1i[k][:, mlo:mlo + cs], s_myv[k],
                                 start=(k == 0), stop=(k == KC - 1))
            nc.vector.tensor_copy(out=s_ai[c][:cs, :], in_=psI[:cs, :])
            # E2 bias: Ai[slot_e2] + w1c * own_flag  (per batch)
            for b in range(B):
                p_e2 = next(sg for sg in segs
                            if sg["kind"] == "in" and sg["b"] == b)["start"]
                nc.vector.tensor_tensor(
                    out=s_aiE2[c][:cs, b:b + 1],
                    in0=s_w1c[:cs, c:c + 1], in1=s_e2f[:cs, b:b + 1],
                    op=ALU.mult)
                nc.vector.tensor_tensor(
                    out=s_aiE2[c][:cs, b:b + 1],
                    in0=s_aiE2[c][:cs, b:b + 1], in1=s_ai[c][:cs, p_e2:p_e2 + 1],
                    op=ALU.add)

        def ts_relu(eng, out, in0, sc):
            if eng is nc.scalar:
                nc.scalar.activation(out, in0, AF.Relu, bias=sc, scale=1.0)
            else:
                eng.tensor_scalar(out=out, in0=in0, scalar1=sc, scalar2=0.0,
                                  op0=ALU.add, op1=ALU.max)

        # estimated per-engine cost (ns) of an N-col relu op, for balancing
        # (HW-measured: DVE 256c ~286ns in 4x mode, ACT 256c ~370ns;
        #  GpSimd tensor_scalar measured ~1.8us -- never give it relu ops)
        def op_cost(eng, n):
            if eng is nc.vector:
                return (210 + n / 4) / 0.96
            return (96 + n) / 0.96

        # ---- main loop over 2-slot tiles ----
        for t in range(ntile):
            hts = []
            for c in range(7):
                cs = CS[c] if c < 6 else 3
                ht = hp[c].tile([cs, HW], BF16, tag=f"ht{c}", name=f"ht{c}_{t}")
                hts.append(ht)
            nc.sync.dma_start(out=hts[6][2:3, :], in_=maskb[:, HW * t:HW * (t + 1)])
            # collect this tile's relu ops, then greedily balance engines.
            # fixed per-tile loads: DVE runs the TT merge+mask chain, ACT the
            # hi-copy and exp, GpSimd the mask TT.
            load = {id(nc.vector): 660.0, id(nc.scalar): 680.0 + 960.0}
            engs = [nc.vector, nc.scalar]
            ops = []   # (ncols, out, in0, scalar, order_group)
            for c in range(7):
                cs = CS[c]
                ht = hts[c]
                for sl in range(2):
                    p = 2 * t + sl
                    b = slot_batch[p]
                    sg = slot_seg[p]
                    base = SLOTW * sl
                    if sg["kind"] != "in":
                        ops.append((256, ht[:cs, base:base + 256],
                                    s_bj0[c][:cs, 512 * b:512 * b + 256],
                                    s_ai[c][:cs, p:p + 1], 0))
                        continue
                    s, e = sg["s"], sg["e"]
                    kk = p - sg["start"]
                    B0 = s + 8 * kk
                    bw = 8 + (B0 & 1)
                    SBe = B0 + bw                    # even static window start
                    Aw = min(SBe + (SBe & 1), 256)   # segA prefix end (even)
                    # segA: [0, Aw) from Bj0 (covers j < i for every core)
                    ops.append((Aw, ht[:cs, base:base + Aw],
                                s_bj0[c][:cs, 512 * b:512 * b + Aw],
                                s_ai[c][:cs, p:p + 1], 0))
                    # band: [B0, B0+bw) from the precomputed per-core source
                    ops.append((bw, ht[:cs, base + B0:base + SBe],
                                s_band[c][:cs, 12 * p:12 * p + bw],
                                s_ai[c][:cs, p:p + 1], 1))
                    # static suffix [SBe, 256) from BjE (== Bj0 past e)
                    if SBe < 256:
                        ops.append((256 - SBe, ht[:cs, base + SBe:base + 256],
                                    s_bje[c][:cs, 512 * b + SBe:512 * b + 256],
                                    s_ai[c][:cs, p:p + 1], 2))
            # issue in order-group order so overwrites are sequenced, engine
            # chosen greedily by estimated completion time
            ops.sort(key=lambda o: o[4])
            for (ncol, o_ap, i_ap, sc, grp) in ops:
                eng = min(engs, key=lambda en: load[id(en)] + op_cost(en, ncol))
                load[id(eng)] += op_cost(eng, ncol)
                ts_relu(eng, o_ap, i_ap, sc)
            # E2 pixel: slot holding row s_b gets +2*w1c at j=e
            for c in range(7):
                cs = CS[c]
                for sl in range(2):
                    p = 2 * t + sl
                    sg = slot_seg[p]
                    if sg["kind"] == "in" and p == sg["start"]:
                        b, e = sg["b"], sg["e"]
                        base = SLOTW * sl
                        ts_relu(nc.vector, hts[c][:cs, base + e:base + e + 1],
                                s_bje[c][:cs, 512 * b + e:512 * b + e + 1],
                                s_aiE2[c][:cs, b:b + 1])

            # matmul: 6 chunks col-packed into two PE column groups + tail
            rhs6 = [hts[c][:, :].rearrange("p (s w) -> p s w", w=SLOTW)
                    [:, :, 0:L] for c in range(6)]
            rhs_t = hts[6][:, :].rearrange("p (s w) -> p s w", w=SLOTW)[:, :, 0:L]
            if COLPACK:
                ps = main_ps.tile([128, 2 * 512], FP32, tag="ps", name=f"ps{t}")
                lo = ps[0:NLAB, 0:2 * L]
                hi = ps[64:64 + NLAB, 512:512 + 2 * L]
                for c in range(6):
                    col = 64 * (c % 2)
                    nc.tensor.matmul(lo if col == 0 else hi,
                                     s_w2[:, NLAB * c:NLAB * (c + 1)], rhs6[c],
                                     start=(c < 2), stop=(c == 5),
                                     tile_position=(0, col))
                nc.tensor.matmul(lo, s_w2t, rhs_t, start=False, stop=True)
                # merge the two half-sums -> fp16 logits (engines can read at
                # most one PSUM operand: stage the hi half through SBUF on ACT)
                hi_sb = mgp.tile([NLAB, 2 * L], FP32, tag="hisb", name=f"hisb{t}")
                nc.scalar.activation(hi_sb, hi, AF.Copy)
                mg = mgp.tile([NLAB, 2 * L], FP16, tag="mg", name=f"mg{t}")
                nc.vector.tensor_tensor(out=mg, in0=lo, in1=hi_sb, op=ALU.add)
            else:
                ps = main_ps.tile([NLAB, 2 * L], FP32, tag="ps", name=f"ps{t}")
                for c in range(6):
                    nc.tensor.matmul(ps, s_w2[:, NLAB * c:NLAB * (c + 1)], rhs6[c],
                                     start=(c == 0), stop=False)
                nc.tensor.matmul(ps, s_w2t, rhs_t, start=False, stop=True)
                mg = ps

            # mask -> exact zeros for invalid pairs (fp16), then exp+sum;
            # invalid pairs contribute exp(0)=1 exactly like the reference
            seg = s_out[:, 2 * L * t:2 * L * (t + 1)]
            mask_eng = nc.gpsimd if COLPACK else nc.vector
            mask_eng.tensor_tensor(out=seg, in0=mg,
                                   in1=s_mrep[:, 2 * L * t:2 * L * (t + 1)],
                                   op=ALU.mult)
            esc = esp.tile([NLAB, 2 * L], FP16, tag="esc", name=f"esc{t}")
            nc.scalar.activation(esc, seg, AF.Exp, accum_out=s_acc[:, t:t + 1])

        # ---- LSE: per-batch sums + AllGather + local tree + log ----
        for b in range(B):
            rs = btiles[b]
            runs = []
            st = rs[0]
            for a, bb in zip(rs, rs[1:] + [None]):
                if bb != a + 1:
                    runs.append((st, a))
                    st = bb
            t0, t1 = runs[0]
            nc.vector.tensor_reduce(out=s_sums[:, b:b + 1],
                                    in_=s_acc[:, t0:t1 + 1],
                                    axis=mybir.AxisListType.X, op=ALU.add)
            for (u0, u1) in runs[1:]:
                tmp = fp.tile([NLAB, 1], FP32, tag=f"tr{b}", name=f"tr{b}_{u0}")
                nc.vector.tensor_reduce(out=tmp, in_=s_acc[:, u0:u1 + 1],
                                        axis=mybir.AxisListType.X, op=ALU.add)
                nc.vector.tensor_tensor(out=s_sums[:, b:b + 1],
                                        in0=s_sums[:, b:b + 1], in1=tmp,
                                        op=ALU.add)
        # correct for padding slots (each contributed 252 * exp(0))
        nc.vector.tensor_tensor(out=s_sums, in0=s_sums, in1=s_cnt2, op=ALU.add)
        cc_in = dram.tile([NLAB, B], FP32, name="cc_in")
        cc_out = nc.dram_tensor("cc_out", [NC, NLAB, B], FP32, kind="Internal",
                                addr_space="Shared").ap()
        nc.sync.dma_start(out=cc_in, in_=s_sums)
        nc.gpsimd.collective_compute(
            "AllGather", ALU.bypass, replica_groups=[list(range(NC))],
            ins=[cc_in[:]], outs=[cc_out], cc_dim="Partition")
        nc.sync.dma_start(out=s_gath.rearrange("k (r c) -> k r c", c=B),
                          in_=cc_out.rearrange("r k c -> k r c"))
        # tree-reduce the 8 rank contributions: cols (r*B + c)
        nc.vector.tensor_tensor(out=s_gath[:, 0:8], in0=s_gath[:, 0:8],
                                in1=s_gath[:, 8:16], op=ALU.add)
        nc.vector.tensor_tensor(out=s_gath[:, 0:4], in0=s_gath[:, 0:4],
                                in1=s_gath[:, 4:8], op=ALU.add)
        nc.vector.tensor_tensor(out=s_gath[:, 0:2], in0=s_gath[:, 0:2],
                                in1=s_gath[:, 2:4], op=ALU.add)
        nc.sync.dma_start(out=lseo, in_=s_gath[:, 0:2])
        nc.scalar.activation(s_nlse, s_gath[:, 0:2], AF.Ln)
        nc.vector.tensor_scalar(out=s_nlse, in0=s_nlse, scalar1=-1.0,
                                scalar2=None, op0=ALU.mult)

        # ---- final: out - LSE over the 4 contiguous batch runs, store ----
        allruns = []
        for b in range(B):
            rs = btiles[b]
            st = rs[0]
            for a, bb in zip(rs, rs[1:] + [None]):
                if bb != a + 1:
                    allruns.append((b, st, a))
                    st = bb
        for i, (b, u0, u1) in enumerate(allruns):
            seg = s_out[:, 2 * L * u0:2 * L * (u1 + 1)]
            nc.vector.tensor_scalar(out=seg, in0=seg,
                                    scalar1=s_nlse[:, b:b + 1], scalar2=None,
                                    op0=ALU.add)
            dmae[i % 2].dma_start(out=outd[:, 2 * L * u0:2 * L * (u1 + 1)],
                                  in_=seg)

    return kern


def kernel(**inputs) -> np.ndarray:
    hidden = np.asarray(inputs["hidden"], dtype=np.float32)
    pred_spans = np.asarray(inputs["pred_spans"]).astype(np.int64)
    span_mask = np.asarray(inputs["span_mask"]).astype(np.int32)
    W1 = np.asarray(inputs["W1"], dtype=np.float32)
    b1 = np.asarray(inputs["b1"], dtype=np.float32)
    W2 = np.asarray(inputs["W2"], dtype=np.float32)
    b2 = np.asarray(inputs["b2"], dtype=np.float32)

    spans = [(int(pred_spans[b, 0]), int(pred_spans[b, 1])) for b in range(B)]
    segs, nslot = plan_slots(spans)
    ntile = nslot // 2

    vecs = hidden[:, 1:L + 1, :]                       # [B, L, 768]
    vecsT = np.concatenate([vecs[0].T, vecs[1].T], axis=1)   # [768, 504]
    W1T = W1.T                                          # [1537, 770]
    w1iT = np.ascontiguousarray(W1T[0:HID]).astype(np.float32)
    w1jT = np.ascontiguousarray(W1T[HID:2 * HID]).astype(np.float32)
    w1c = np.ascontiguousarray(W1T[2 * HID]).astype(np.float32)
    W2T = np.ascontiguousarray(W2.T)                    # [770, 36]
    w2T6 = W2T[0:HID]
    w2Tt = np.stack([W2T[768], W2T[769], b2], axis=0)   # [3, 36]

    maskf_full = span_mask.astype(np.float32).clip(0, 1)
    nin_tot = sum(sg["nslots"] for sg in segs if sg["kind"] == "in")

    in_maps = []
    slot_maps = []
    for c in range(NC):
        sm = slot_map_for_core(segs, nslot, c)
        slot_maps.append(sm)
        myv = np.zeros((HID, nslot), np.float32)
        maskrep = np.zeros(nslot * L, np.float32)
        npad = [0, 0]
        for p, ent in enumerate(sm):
            if ent is None:
                npad[next(sg["b"] for sg in segs
                          if sg["start"] <= p < sg["start"] + sg["nslots"])] += 1
                continue
            b, r = ent
            myv[:, p] = vecs[b, r]
            maskrep[L * p:L * (p + 1)] = maskf_full[r]
        maskb = np.zeros((1, nslot * 384), np.float32)
        for p, ent in enumerate(sm):
            if ent is not None:
                o = 384 * p
                maskb[0, o:o + L] = maskf_full[ent[1]]
        cnt2 = np.zeros((NLAB, B), np.float32)
        for b in range(B):
            cnt2[:, b] = -float(L * npad[b])
        stepm = np.zeros((128, 12 * nin_tot), np.float32)
        for sg in segs:
            if sg["kind"] != "in":
                continue
            s, e = sg["s"], sg["e"]
            for kk in range(sg["nslots"]):
                q = sg["start"] + kk
                B0 = s + 8 * kk
                for d in range(12):
                    if d >= c and B0 + d <= e:
                        stepm[:, 12 * q + d] = 1.0
        e2f = np.zeros((128, 2), np.float32)
        for b in range(B):
            if c == 0:
                e2f[:, b] = 1.0      # row s_b lives on core 0 (slot seg start)
        in_maps.append({
            "w1iT": w1iT, "w1jT": w1jT, "w1c": w1c, "b1": b1,
            "w2T6": w2T6, "w2Tt": w2Tt,
            "vecsT": vecsT, "myvT": myv,
            "maskb": maskb, "maskrep": maskrep, "stepm": stepm, "cnt2": cnt2,
            "e2f": e2f,
        })

    # ---- build program ----
    nc = bacc.Bacc("TRN2", target_bir_lowering=False, debug=False,
                   enable_asserts=False, num_devices=NC)

    def mk(name, arr, dt):
        return nc.dram_tensor(name, list(arr.shape), dt, kind="ExternalInput").ap()

    ml_bf = lambda n, a: mk(n, a, BF16)
    ml_f16 = lambda n, a: mk(n, a, FP16)
    ml_f32 = lambda n, a: mk(n, a, FP32)
    ex = in_maps[0]
    ins_aps = {
        "w1iT": ml_bf("w1iT", ex["w1iT"]), "w1jT": ml_bf("w1jT", ex["w1jT"]),
        "w1c": ml_f32("w1c", ex["w1c"]), "b1": ml_f32("b1", ex["b1"]),
        "w2T6": ml_bf("w2T6", ex["w2T6"]), "w2Tt": ml_bf("w2Tt", ex["w2Tt"]),
        "vecsT": ml_bf("vecsT", ex["vecsT"]), "myvT": ml_bf("myvT", ex["myvT"]),
        "maskb": ml_bf("maskb", ex["maskb"]),
        "maskrep": ml_f16("maskrep", ex["maskrep"]),
        "stepm": ml_bf("stepm", ex["stepm"]),
        "cnt2": ml_f32("cnt2", ex["cnt2"]), "e2f": ml_f32("e2f", ex["e2f"]),
    }
    outs_aps = {
        "out": nc.dram_tensor("out", [NLAB, nslot * L], FP16,
                              kind="ExternalOutput").ap(),
        "lse": nc.dram_tensor("lse", [NLAB, B], FP32,
                              kind="ExternalOutput").ap(),
    }

    kern = build_kernel(spans, segs, nslot)
    with tile.TileContext(nc) as t:
        kern(t, outs_aps, ins_aps)
    nc.compile()

    def cast_maps(m):
        out = {}
        for k, v in m.items():
            dt = ins_aps[k].dtype
            out[k] = v.astype(mybir.dt.np(dt))
        return out

    in_maps_c = [cast_maps(m) for m in in_maps]

    if os.environ.get("BK_BUILD_ONLY"):
        print("BUILD OK")
        return np.zeros((B, NLAB, L * L), np.float32)

    if os.environ.get("BK_SIM"):
        from concourse.bass_interp import MultiCoreSim

        sim = MultiCoreSim(nc, num_cores=NC, require_finite=False,
                           require_nnan=False)
        for c, cs in sim.cores.items():
            for name, arr in in_maps_c[c].items():
                cs.tensor(name)[:] = arr
            if nc.partition_id_tensor is not None:
                cs.tensor(nc.partition_id_tensor.name)[:] = np.array(
                    [[c]], dtype=np.uint32)
        sim.simulate(check_with_hw=False)

        class _R:
            results = [{"out": np.asarray(sim.cores[c].tensor("out")),
                        "lse": np.asarray(sim.cores[c].tensor("lse"))}
                       for c in range(NC)]
        res = _R()
    else:
        trace = bool(int(os.environ.get("BK_TRACE", "0")))
        res = run_bass_kernel_spmd(nc, in_maps_c, core_ids=list(range(NC)),
                                   trace=trace)
        if trace and res.exec_time_ns is not None:
            print(f"HW exec time: {res.exec_time_ns} ns")

    # ---- unshard ----
    out_full = np.zeros((B, NLAB, L * L), np.float32)
    for c in range(NC):
        oc = np.asarray(res.results[c]["out"]).astype(np.float32)
        for p, ent in enumerate(slot_maps[c]):
            if ent is None:
                continue
            b, r = ent
            out_full[b, :, L * r:L * (r + 1)] = oc[:, L * p:L * (p + 1)]
    return out_full
